# revision 34
# baseline (speedup 1.0000x reference)
"""Trainium2 Bass kernel for nn_CrossVariableMixingHydra.

Math (per batch b, x kept in native (L, C) layout, l = partitions):
  hl[r, c]  = sum_l Wd[r, l] x[l, c] + bd[r]          # down-projection
  qk[s, c]  = [Wq.T | Wk.T] rows . hl  (s: 0-63 Q, 64-127 K)
  vg[s, c]  = [Wv.T | Wg.T] rows . hl
  gate      = sigmoid(vg[64:] + bg)
  rstd{Q,K}[c] = 1/sqrt(sum_s qk[s,c]^2 + eps^2)      # l2norm denominators
  gf[s]     = sum_c K̂ V = sum_c qk[64+s,c] rstdK[c] vg[s,c]
  ha[s, c]  = Q̂ gf gate = qk[s,c] rstdQ[c] gf[s] gate[s,c]
  v[l, c]   = x[l, c] + a Wu[l,:] ha[:,c] + a bu[l]   # residual
  y[l, c]   = (v - mean_c v) rstd_c(v)                # LayerNorm over C

Strategy: data-parallel over batch on 8 cores (8 batches/core), software-
pipelined per core. Engine balance per batch (16 l-chunks of (128, 512)):
  DVE : qk2 square + attention chain (kv/gfs/at) + P1 (v = up+bu+x, accum
        sums) x16 + LN stats smalls
  ACT : hle bias-cast + gate + nrm/sd + P2 (Square, accum sums2) for
        P2_ACT chunks
  Pool: P2 for the rest + P3 (y = v*rstd + nmr) x16 + ha gate-mult
  PE  : all matmuls (cheap)
Loads prefetch on the sync (SP) HWDGE ring; stores go on the scalar (ACT)
HWDGE ring at iteration end so they never block load prefetch. vt (the
residual) is held bf16 (error ~2e-3 vs the 2e-2 budget); the LN stats are
accumulated in f32.
"""

import numpy as np
import ml_dtypes

import concourse.bass as bass
import concourse.mybir as mybir
import concourse.tile as tile
import concourse.bass_utils as bass_utils

B, L, C, R = 64, 2048, 512, 64
N_CORES = 8
BPC = B // N_CORES          # batches per core
NCH = L // 128              # l-chunks per batch
EPS_LN = 1e-5
EPS_NORM_SQ = 1e-24         # (1e-12)^2 : sqrt(ssq + eps^2) == max(sqrt(ssq), 1e-12)

P2_ACT = 12                 # square-accum chunks on ACT (rest on Pool)

f32 = mybir.dt.float32
f32r = mybir.dt.float32r
bf16 = mybir.dt.bfloat16
Alu = mybir.AluOpType
Act = mybir.ActivationFunctionType


def _split_waits(nc, max_waits: int = 1):
    """This container's walrus build rejects instructions carrying more than
    one sync wait. Move excess waits onto preceding NoOps on the same engine
    (engines execute in order, so semantics are unchanged)."""
    for f in nc.m.functions:
        for bb in f.blocks:
            insts = bb.instructions
            i = 0
            while i < len(insts):
                inst = insts[i]
                si = inst.sync_info
                if si is not None and si.on_wait and len(si.on_wait) > max_waits:
                    waits = list(si.on_wait)
                    si.on_wait = waits[:max_waits]
                    extra = waits[max_waits:]
                    nops = []
                    for j in range(0, len(extra), max_waits):
                        nop = mybir.InstNoOp(name=f"{inst.name}-ws{j}", ins=[], outs=[])
                        nop.engine = inst.engine
                        nop.sync_info = mybir.SyncInfo(
                            on_wait=extra[j : j + max_waits], on_update=[]
                        )
                        nops.append(nop)
                    for k, nop in enumerate(nops):
                        insts.insert(i + k, nop)
                    i += len(nops)
                i += 1
    return nc


def build(trivial_affine: bool, repeat: int = 1, probe: str = "", p2_dve: int = 1,
          store_lag: int = 1, ident: bool = False, lgrp: int = 16, sgrp: int = 4,
          b_first: bool = False, load_ring: str = "sync", ybufs: int = 2, stgrp: int = 8):
    """probe='dma' keeps only the x-load and out-store DMAs; probe='nostore'
    drops the out stores. Timing probes only — results garbage."""
    nc = bass.Bass("TRN2", target_bir_lowering=False, debug=False, num_devices=1)

    x_ap = nc.dram_tensor("x", [BPC, L, C], f32r, kind="ExternalInput").ap()
    wdt_ap = nc.dram_tensor("wdt", [L, R], f32r, kind="ExternalInput").ap()
    bd_ap = nc.dram_tensor("bdc", [R, 1], f32, kind="ExternalInput").ap()
    bg_ap = nc.dram_tensor("bgc", [R, 1], f32, kind="ExternalInput").ap()
    wqk_ap = nc.dram_tensor("wqk", [R, 128], bf16, kind="ExternalInput").ap()
    wvg_ap = nc.dram_tensor("wvg", [R, 128], bf16, kind="ExternalInput").ap()
    wut_ap = nc.dram_tensor("wut", [R, L], bf16, kind="ExternalInput").ap()
    buc_ap = nc.dram_tensor("buc", [128, NCH], f32, kind="ExternalInput").ap()
    idn_ap = nc.dram_tensor("idn", [128, 128], f32r, kind="ExternalInput").ap()
    selm_ap = nc.dram_tensor("selm", [128, 2], bf16, kind="ExternalInput").ap()
    epsn_ap = nc.dram_tensor("epsn", [2, 1], f32, kind="ExternalInput").ap()
    epsl_ap = nc.dram_tensor("epsl", [128, 1], f32, kind="ExternalInput").ap()
    sel2_ap = nc.dram_tensor("sel2", [2, 128], f32, kind="ExternalInput").ap()
    if not trivial_affine:
        gam_ap = nc.dram_tensor("gamb", [128, C], f32, kind="ExternalInput").ap()
        bet_ap = nc.dram_tensor("betb", [128, C], f32, kind="ExternalInput").ap()
    out_ap = nc.dram_tensor("out", [BPC, L, C], f32, kind="ExternalOutput").ap()

    LGRP = lgrp              # l-chunks per load DMA
    SGRP = sgrp              # stats group
    STGRP = stgrp            # l-chunks per store DMA

    with tile.TileContext(nc) as tc:
        with (
            tc.tile_pool(name="consts", bufs=1) as cp,
            tc.tile_pool(name="xp", bufs=3) as xp,
            tc.tile_pool(name="ys", bufs=ybufs) as yp,
            tc.tile_pool(name="attn", bufs=2) as ap_,
            tc.tile_pool(name="attn1", bufs=2) as ap1,
            tc.tile_pool(name="vs", bufs=2) as vp,
            tc.tile_pool(name="sq", bufs=1) as qp,
            tc.tile_pool(name="st", bufs=2) as sp,
            tc.tile_pool(name="ps_hl", bufs=1, space="PSUM") as ps_hl,
            tc.tile_pool(name="ps_qk", bufs=2, space="PSUM") as ps_qk,
            tc.tile_pool(name="ps_vg", bufs=2, space="PSUM") as ps_vg,
            tc.tile_pool(name="ps_sqbc", bufs=1, space="PSUM") as ps_sqbc,
            tc.tile_pool(name="ps_up", bufs=2, space="PSUM") as ps_up,
        ):
            # --- constants (loaded once) ---
            wdt = cp.tile([128, NCH * R], f32r)
            nc.sync.dma_start(
                wdt[:].rearrange("p (n r) -> p n r", n=NCH),
                wdt_ap[:].rearrange("(n p) r -> p n r", p=128),
            )
            bd = cp.tile([R, 1], f32)
            nc.sync.dma_start(bd[:], bd_ap[:])
            bg = cp.tile([R, 1], f32)
            nc.sync.dma_start(bg[:], bg_ap[:])
            wqk = cp.tile([R, 128], bf16)
            nc.sync.dma_start(wqk[:], wqk_ap[:])
            wvg = cp.tile([R, 128], bf16)
            nc.sync.dma_start(wvg[:], wvg_ap[:])
            wut = cp.tile([R, L], bf16)
            nc.sync.dma_start(wut[:], wut_ap[:])
            buc = cp.tile([128, NCH], f32)
            nc.sync.dma_start(buc[:], buc_ap[:])
            idn = cp.tile([128, 128], f32r)
            nc.sync.dma_start(idn[:], idn_ap[:])
            selm = cp.tile([128, 2], bf16)
            nc.sync.dma_start(selm[:], selm_ap[:])
            sel2 = cp.tile([2, 128], f32)
            nc.sync.dma_start(sel2[:], sel2_ap[:])
            epsn = cp.tile([2, 1], f32)
            nc.sync.dma_start(epsn[:], epsn_ap[:])
            epsl = cp.tile([128, 1], f32)
            nc.sync.dma_start(epsl[:], epsl_ap[:])
            if not trivial_affine:
                gam = cp.tile([128, C], f32)
                nc.sync.dma_start(gam[:], gam_ap[:])
                bet = cp.tile([128, C], f32)
                nc.sync.dma_start(bet[:], bet_ap[:])

            def emit_load(b):
                ring = nc.sync if load_ring == "sync" else nc.scalar
                xb = xp.tile([128, NCH * C], f32r, tag="xb")
                for g in range(NCH // LGRP):
                    ring.dma_start(
                        xb[:, g * LGRP * C : (g + 1) * LGRP * C].rearrange(
                            "p (n c) -> p n c", n=LGRP
                        ),
                        x_ap[b, g * LGRP * 128 : (g + 1) * LGRP * 128, :].rearrange(
                            "(n p) c -> p n c", p=128
                        ),
                    )
                return xb

            def emit_store_probe(b, xb):
                for g in range(NCH // STGRP):
                    nc.sync.dma_start(
                        out_ap[b, g * STGRP * 128 : (g + 1) * STGRP * 128, :].rearrange(
                            "(n p) c -> p n c", p=128
                        ),
                        xb[:, g * STGRP * C : (g + 1) * STGRP * C]
                        .bitcast(f32)
                        .rearrange("p (n c) -> p n c", n=STGRP),
                    )

            def stage_a_head(b, xb):
                """down-projection + QKVG matmuls + short-dep ACT/DVE ops."""
                hl_ps = ps_hl.tile([R, C], f32, tag="hl")
                for k in range(NCH):
                    nc.tensor.matmul(
                        hl_ps[:],
                        wdt[:, k * R : (k + 1) * R],
                        xb[:, k * C : (k + 1) * C],
                        start=(k == 0),
                        stop=(k == NCH - 1),
                    )
                hle = ap_.tile([R, C], bf16, tag="hle")
                nc.scalar.activation(hle[:], hl_ps[:], Act.Identity, bias=bd[:], scale=1.0)

                qk_ps = ps_qk.tile([128, C], f32, tag="qk")
                nc.tensor.matmul(qk_ps[:], wqk[:], hle[:], start=True, stop=True)
                vg_ps = ps_vg.tile([128, C], f32, tag="vg")
                nc.tensor.matmul(vg_ps[:], wvg[:], hle[:], start=True, stop=True)

                qk_s = ap_.tile([128, C], bf16, tag="qks")
                nc.scalar.copy(qk_s[:], qk_ps[:])
                gate = ap_.tile([R, C], bf16, tag="gate")
                nc.scalar.activation(gate[:], vg_ps[R:128, :], Act.Sigmoid, bias=bg[:])
                qk2 = ap1.tile([128, C], bf16, tag="qk2")
                nc.scalar.activation(qk2[:], qk_s[:], Act.Square)
                ssq_fl = ps_sqbc.tile([128, C], f32, tag="sqbc", name="ssq_fl")
                ssq_ps = ssq_fl[0:2, :]
                nc.tensor.matmul(ssq_ps[:], selm[:], qk2[:], start=True, stop=True)
                nrm = sp.tile([2, C], f32, tag="nrm")
                nc.scalar.activation(nrm[:], ssq_ps[:], Act.Sqrt, bias=epsn[:])
                rstd2 = sp.tile([2, C], f32, tag="rstd2")
                nc.vector.reciprocal(rstd2[:], nrm[:])
                bc_ps = ps_sqbc.tile([128, C], f32, tag="sqbc")
                nc.tensor.matmul(bc_ps[:], sel2[:], rstd2[:], start=True, stop=True)
                return (qk_s, vg_ps, gate, bc_ps)

            def stage_a_tail(b, head):
                """the serial attention chain: runs in each engine's tail slack."""
                qk_s, vg_ps, gate, bc_ps = head

                kv = ap1.tile([R, C], f32, tag="kv")
                nc.vector.tensor_tensor(kv[:], vg_ps[0:R, :], qk_s[R:128, :], op=Alu.mult)
                gf = sp.tile([R, 1], f32, tag="gf")
                nc.vector.scalar_tensor_tensor(
                    kv[:], bc_ps[R:128, :], 1.0, kv[:], op0=Alu.mult, op1=Alu.mult,
                    accum_out=gf[:],
                )
                at = ap1.tile([R, C], f32, tag="at")
                nc.vector.scalar_tensor_tensor(
                    at[:], bc_ps[0:R, :], gf[:], qk_s[0:R, :], op0=Alu.mult, op1=Alu.mult
                )
                ha = ap_.tile([R, C], bf16, tag="ha")
                nc.gpsimd.tensor_tensor(ha[:], at[:], gate[:], op=Alu.mult)
                return ha

            def stage_b(b, xb, ha):
                """upmix + residual + LN (stats per 4-chunk group) + store."""

                sums = sp.tile([128, NCH], f32, tag="sums")
                sums2 = sp.tile([128, NCH], f32, tag="sums2")
                if probe == "nop2":
                    nc.gpsimd.memset(sums2[:], 1.0)
                nmu2 = sp.tile([128, NCH], f32, tag="nmu2")
                var = sp.tile([128, NCH], f32, tag="var")
                sd = sp.tile([128, NCH], f32, tag="sd")
                rstd = sp.tile([128, NCH], f32, tag="rstd")
                nmr = sp.tile([128, NCH], f32, tag="nmr")
                vt = vp.tile([128, NCH * C], bf16, tag="v")
                ygs = [None]
                yg_g0 = [0]
                sq_a = qp.tile([128, C], bf16, tag="sqa")
                sq_p = qp.tile([128, C], bf16, tag="sqp")
                for g in range(NCH // SGRP):
                    g0, g1 = g * SGRP, (g + 1) * SGRP
                    for j in range(SGRP):
                        k = g0 + j
                        up_ps = ps_up.tile([128, C], f32, tag="up")
                        nc.tensor.matmul(
                            up_ps[:], wut[:, k * 128 : (k + 1) * 128], ha[:],
                            start=True, stop=not ident,
                        )
                        if ident:
                            nc.tensor.matmul(
                                up_ps[:], idn[:], xb[:, k * C : (k + 1) * C],
                                start=False, stop=True,
                            )
                        vk = vt[:, k * C : (k + 1) * C]
                        if ident:
                            nc.vector.tensor_scalar(
                                vk, up_ps[:], buc[:, k : k + 1], 0.0,
                                op0=Alu.add, op1=Alu.add,
                                accum_out=sums[:, k : k + 1],
                            )
                        else:
                            nc.vector.scalar_tensor_tensor(
                                vk, up_ps[:], buc[:, k : k + 1],
                                xb[:, k * C : (k + 1) * C].bitcast(f32),
                                op0=Alu.add, op1=Alu.add,
                                accum_out=sums[:, k : k + 1],
                            )
                        if probe == "nop2":
                            pass
                        elif j < SGRP - p2_dve:
                            nc.scalar.activation(
                                sq_a[:], vk, Act.Square, accum_out=sums2[:, k : k + 1]
                            )
                        else:
                            nc.vector.scalar_tensor_tensor(
                                sq_p[:], vk, 1.0, vk, op0=Alu.mult, op1=Alu.mult,
                                accum_out=sums2[:, k : k + 1],
                            )
                    # --- group LN stats: var = E[v^2] - mu^2, rstd, -mu*rstd ---
                    nc.vector.scalar_tensor_tensor(
                        nmu2[:, g0:g1], sums[:, g0:g1], -1.0 / (C * C), sums[:, g0:g1],
                        op0=Alu.mult, op1=Alu.mult,
                    )
                    nc.vector.scalar_tensor_tensor(
                        var[:, g0:g1], sums2[:, g0:g1], 1.0 / C, nmu2[:, g0:g1],
                        op0=Alu.mult, op1=Alu.add,
                    )
                    nc.scalar.activation(sd[:, g0:g1], var[:, g0:g1], Act.Sqrt, bias=epsl[:])
                    nc.vector.reciprocal(rstd[:, g0:g1], sd[:, g0:g1])
                    nc.vector.scalar_tensor_tensor(
                        nmr[:, g0:g1], sums[:, g0:g1], -1.0 / C, rstd[:, g0:g1],
                        op0=Alu.mult, op1=Alu.mult,
                    )

                    # --- finalize y = (v - mu) * rstd [* gamma + beta] ---
                    if g % (STGRP // SGRP) == 0:
                        yg = yp.tile([128, STGRP * C], f32, tag="y")
                        ygs[0] = yg
                        yg_g0[0] = g0
                    yg = ygs[0]
                    for j in range(SGRP):
                        k = g0 + j
                        jo = k - yg_g0[0]
                        ys = yg[:, jo * C : (jo + 1) * C]
                        nc.gpsimd.tensor_scalar(
                            ys, vt[:, k * C : (k + 1) * C],
                            rstd[:, k : k + 1], nmr[:, k : k + 1],
                            op0=Alu.mult, op1=Alu.add,
                        )
                        if not trivial_affine:
                            nc.vector.tensor_tensor(ys, ys, gam[:], op=Alu.mult)
                            nc.gpsimd.tensor_tensor(ys, ys, bet[:], op=Alu.add)
                    if probe != "nostore" and g1 == yg_g0[0] + STGRP:
                        def _st(s0=yg_g0[0], yg=yg):
                            nc.sync.dma_start(
                                out_ap[b, s0 * 128 : (s0 + STGRP) * 128, :].rearrange(
                                    "(n p) c -> p n c", p=128
                                ),
                                yg[:].rearrange("p (n c) -> p n c", n=STGRP),
                            )
                        if store_lag:
                            pend_stores.append(_st)
                            if len(pend_stores) > store_lag:
                                pend_stores.pop(0)()
                        else:
                            _st()

            def flush_stores():
                while pend_stores:
                    pend_stores.pop(0)()

            ha_const = cp.tile([R, C], bf16)
            nc.gpsimd.memset(ha_const[:], 0.01)
            pend_stores = []

            def whole_body():
                pend_stores.clear()
                if probe == "chainonly":
                    xbs = [None] * BPC
                    xbs[0] = emit_load(0)
                    if BPC > 1:
                        xbs[1] = emit_load(1)
                    h0 = stage_a_head(0, xbs[0])
                    stage_a_tail(0, h0)
                    for b in range(1, BPC):
                        if b + 1 < BPC:
                            xbs[b + 1] = emit_load(b + 1)
                        hb = stage_a_head(b, xbs[b])
                        stage_a_tail(b, hb)
                    return
                if probe == "bulkonly":
                    xbs = [None] * BPC
                    xbs[0] = emit_load(0)
                    if BPC > 1:
                        xbs[1] = emit_load(1)
                    for b in range(1, BPC):
                        if b + 1 < BPC:
                            xbs[b + 1] = emit_load(b + 1)
                        stage_b(b - 1, xbs[b - 1], ha_const)
                    stage_b(BPC - 1, xbs[BPC - 1], ha_const)
                    flush_stores()
                    return
                if probe == "dma":
                    for b in range(BPC):
                        xb = emit_load(b)
                        emit_store_probe(b, xb)
                    return
                xbs = [None] * BPC
                has = [None] * BPC
                heads = [None] * BPC
                xbs[0] = emit_load(0)
                if BPC > 1:
                    xbs[1] = emit_load(1)
                heads[0] = stage_a_head(0, xbs[0])
                has[0] = stage_a_tail(0, heads[0])
                for b in range(1, BPC):
                    if b + 1 < BPC:
                        xbs[b + 1] = emit_load(b + 1)
                    ha_use = ha_const if probe == "hagate" else has[b - 1]
                    if b_first:
                        stage_b(b - 1, xbs[b - 1], ha_use)
                        heads[b] = stage_a_head(b, xbs[b])
                    else:
                        heads[b] = stage_a_head(b, xbs[b])
                        stage_b(b - 1, xbs[b - 1], ha_use)
                    has[b] = stage_a_tail(b, heads[b])
                stage_b(BPC - 1, xbs[BPC - 1],
                        ha_const if probe == "hagate" else has[BPC - 1])
                flush_stores()
            if repeat == 1:
                whole_body()
            elif repeat < 0:
                for _ in range(-repeat):   # unrolled (for simulation)
                    whole_body()
            else:
                with tc.For_i(0, repeat, 1):
                    whole_body()

    return _split_waits(nc)


def prep_inputs(x, Wd, bd, Wq, Wk, Wv, Wg, bg, Wu, bu, gamma, beta, alpha):
    x = np.ascontiguousarray(np.asarray(x, dtype=np.float32))
    Wd = np.asarray(Wd, np.float32); bd = np.asarray(bd, np.float32)
    Wq = np.asarray(Wq, np.float32); Wk = np.asarray(Wk, np.float32)
    Wv = np.asarray(Wv, np.float32); Wg = np.asarray(Wg, np.float32)
    bg = np.asarray(bg, np.float32)
    Wu = np.asarray(Wu, np.float32); bu = np.asarray(bu, np.float32)
    gamma = np.asarray(gamma, np.float32); beta = np.asarray(beta, np.float32)
    alpha = np.float32(np.asarray(alpha))

    trivial = bool(np.all(gamma == 1.0) and np.all(beta == 0.0))

    wdt = np.ascontiguousarray(Wd.T)                       # (L, R) f32
    bdc = np.ascontiguousarray(bd[:, None])                # (R, 1)
    bgc = np.ascontiguousarray(bg[:, None])                # (R, 1)
    wqk = np.zeros((R, 128), np.float32)
    wqk[:, 0:R] = Wq.T
    wqk[:, R:128] = Wk.T
    wvg = np.zeros((R, 128), np.float32)
    wvg[:, 0:R] = Wv.T
    wvg[:, R:128] = Wg.T
    wut = np.ascontiguousarray(alpha * Wu.T)               # (R, L)
    buc = np.ascontiguousarray((alpha * bu).reshape(NCH, 128).T)  # (128, NCH)
    selm = np.zeros((128, 2), np.float32)
    selm[0:R, 0] = 1.0
    selm[R:128, 1] = 1.0
    sel2 = np.zeros((2, 128), np.float32)
    sel2[0, 0:R] = 1.0
    sel2[1, R:128] = 1.0

    common = dict(
        idn=np.eye(128, dtype=np.float32),
        wdt=wdt,
        bdc=bdc,
        bgc=bgc,
        wqk=wqk.astype(ml_dtypes.bfloat16),
        wvg=wvg.astype(ml_dtypes.bfloat16),
        wut=wut.astype(ml_dtypes.bfloat16),
        buc=buc,
        selm=selm.astype(ml_dtypes.bfloat16),
        sel2=sel2,
        epsn=np.full((2, 1), EPS_NORM_SQ, np.float32),
        epsl=np.full((128, 1), EPS_LN, np.float32),
    )
    if not trivial:
        common["gamb"] = np.ascontiguousarray(np.tile(gamma[None, :], (128, 1)))
        common["betb"] = np.ascontiguousarray(np.tile(beta[None, :], (128, 1)))

    in_maps = []
    for c in range(N_CORES):
        m = dict(common)
        m["x"] = np.ascontiguousarray(x[c * BPC : (c + 1) * BPC])
        in_maps.append(m)
    return in_maps, trivial


_nc_cache = {}


def kernel(**inputs) -> np.ndarray:
    in_maps, trivial = prep_inputs(**inputs)
    if trivial not in _nc_cache:
        _nc_cache[trivial] = build(trivial)
    nc = _nc_cache[trivial]
    res = bass_utils.run_bass_kernel_spmd(nc, in_maps, core_ids=list(range(N_CORES)))
    out = np.concatenate([res.results[c]["out"] for c in range(N_CORES)], axis=0)
    return out.astype(np.float32, copy=False)
